# revision 6
# baseline (speedup 1.0000x reference)
"""DGS2D on 8 TRN2 cores: windowed dma_gather + packed fp16 DVE compute.

Contract: kernel(**inputs) takes the FULL inputs
  input [4,32,512,512] f32, grid [4,65536,3] f32, fScaleWidth/Height [4] f32
and returns the FULL output [4, 32, 4, 65536] f32.

Sharding: 2 cores per batch element, each with half the queries.

vs the 256-indirect-DMA baseline (~3.9x faster, ~128us/core-pass):
  - SWDGE descriptor generation (~9 ns/descriptor on the Q7s) was the
    baseline bottleneck (~370us serial on the Pool engine).  Gathers now
    use dma_gather (one instruction per <=1024 indices -- the SWDGE
    descriptor-ring limit) spread across num_swdge_queues=4 parallel
    descriptor generators: ~83us/pass for 36864 gathered rows.
  - fp16 4-corner stencil table (256B rows) halves gather HBM bytes; it is
    uploaded as 8 window tensors of 32768 rows so each window's int16
    indices address it directly.
  - host sorts queries by pixel row, buckets them per 64-image-row window
    (padded to CAP=4608), and unpermutes the output columns afterwards;
    all sampling arithmetic stays on device (host floor matches device
    fp32 arithmetic op-for-op so row choice and weights always agree).
  - compute is 10 packed DVE ops per 2-window tile; the per-query tx/ty
    operands are materialized by the otherwise-idle ACT engine (stride-0
    broadcast reads are slow on DVE: -25us).
  - outputs staged f32 as [phi,zCam]/[yCam,xCam] pairs and stored via
    HWDGE with 576B-contiguous dram runs; host splits the pairs while
    unpermuting.


vs v3:
  - compute tiles span 2 windows (72 query slots/partition): fewer, larger
    DVE ops.
  - operand packing: V=[ab0,ab1,dd0,dd1] 4C scratch; [dy,e->djx] pair tile;
    [y,x] and [t4,t5] produced by single 2C ops via packed [ay,ax]/[czy,czx]
    setup arrays; 10 DVE ops per 2-window iter (was 28 per 2 windows).
  - packed setup (jx/iy share the 255.5 scale since H==W).
  - fsw/fsh passed as one [1,2] tensor.
"""
import numpy as np

import concourse.bacc as bacc
import concourse.bass as bass
import concourse.mybir as mybir
import concourse.tile as tile

P = 128
F32 = mybir.dt.float32
F16 = mybir.dt.float16
I32 = mybir.dt.int32
I16 = mybir.dt.int16
Alu = mybir.AluOpType

B, C, H, W, Q = 4, 32, 512, 512, 65536
NCORES = 8
QC = Q // (NCORES // B)
NW = 8
WROWS = (H // NW) * W        # 32768
CAP = 4608
QCP = NW * CAP               # 36864
S3 = QCP // P                # 288
SW = CAP // P                # 36
SW2 = 2 * SW                 # 72 (compute tile)
GE = 4 * C                   # 128
HALF = 0.5 * (W - 1)         # == 0.5*(H-1) == 255.5


def _build_kernel(repeat=1, swq=4):
    D2 = 2 * C
    CDT = F16
    GRP = SW2                # store group = one compute iter = 144 cols

    nc = bacc.Bacc("TRN2", target_bir_lowering=False, debug=False,
                   num_swdge_queues=swq)

    featw = [nc.dram_tensor(f"featw{w}", [WROWS, GE], F16,
                            kind="ExternalInput") for w in range(NW)]
    grid_q = nc.dram_tensor("grid_q", [QCP, 3], F32, kind="ExternalInput")
    idx16 = nc.dram_tensor("idx16", [P, QCP // 16], I16, kind="ExternalInput")
    fswh = nc.dram_tensor("fswh", [1, 2], F32, kind="ExternalInput")
    # paired layout: [g, c, q, u] with (g,u) = (0,0)=phi (0,1)=zCam
    #                                  (1,0)=yCam (1,1)=xCam
    # -> 576B contiguous runs per store descriptor (no <512B RMW penalty)
    out = nc.dram_tensor("out", [2, C, QCP, 2], F32,
                         kind="ExternalOutput")

    with tile.TileContext(nc) as tc:
        with (
            tc.tile_pool(name="setup", bufs=1) as sp,
            tc.tile_pool(name="gp", bufs=2) as gp,
            tc.tile_pool(name="ep", bufs=1) as ep,
            tc.tile_pool(name="tp", bufs=1) as tp,
            tc.tile_pool(name="opA", bufs=1) as opA,
            tc.tile_pool(name="opB", bufs=1) as opB,
        ):
            # ---------------- setup ----------------------------------------
            grid_sb = sp.tile([P, S3, 3], F32)
            nc.sync.dma_start(
                grid_sb[:], grid_q[:].rearrange("(p s) t -> p s t", p=P))
            idx_sb = sp.tile([P, QCP // 16], I16)
            nc.sync.dma_start(idx_sb[:], idx16[:])
            fwh_sb = sp.tile([1, 2], F32)
            nc.sync.dma_start(fwh_sb[:], fswh[:])

            yxv = grid_sb[:, :, 0:2]          # [P,S3,2] = [y, x]
            zv = grid_sb[:, :, 2]

            # pixel coords for both axes in one op (half_w == half_h);
            # packed order [iy, jx] -> fractional [ty, tx]
            jxy = sp.tile([P, S3, 2], F32)
            nc.vector.tensor_scalar(out=jxy[:], in0=yxv, scalar1=1.0,
                                    scalar2=HALF, op0=Alu.add, op1=Alu.mult)

            # fractional parts (floor via round + is_gt correction)
            ri = sp.tile([P, S3, 2], I32)
            nc.vector.tensor_copy(ri[:], jxy[:])
            rf = sp.tile([P, S3, 2], F32)
            nc.vector.tensor_copy(rf[:], ri[:])
            mk = sp.tile([P, S3, 2], F32)
            nc.vector.tensor_tensor(out=mk[:], in0=rf[:], in1=jxy[:],
                                    op=Alu.is_gt)
            fl = sp.tile([P, S3, 2], F32)
            nc.vector.tensor_tensor(out=fl[:], in0=rf[:], in1=mk[:],
                                    op=Alu.subtract)
            txy = sp.tile([P, S3, 2], CDT)    # [ty, tx] fp16
            nc.vector.tensor_tensor(out=txy[:], in0=jxy[:], in1=fl[:],
                                    op=Alu.subtract)

            zinv = sp.tile([P, S3], F32)
            nc.vector.reciprocal(zinv[:], zv)

            fwhb = sp.tile([P, 2], F32)      # [fh, fw] (host order)
            nc.gpsimd.partition_broadcast(fwhb[:], fwh_sb[:])
            fhw = sp.tile([P, 2], F32)       # [fh*HALF, fw*HALF]
            nc.vector.tensor_scalar(
                out=fhw[:], in0=fwhb[:],
                scalar1=HALF, scalar2=None, op0=Alu.mult)
            # [ay, ax] = [fh, fw]*HALF/z
            ayx = sp.tile([P, S3, 2], CDT)
            nc.vector.tensor_tensor(
                out=ayx[:],
                in0=zinv[:, :, None].to_broadcast([P, S3, 2]),
                in1=fhw[:, None, :].to_broadcast([P, S3, 2]), op=Alu.mult)
            # [czy, czx] = -HALF*[y, x]/z
            czyx = sp.tile([P, S3, 2], CDT)
            nc.vector.scalar_tensor_tensor(
                out=czyx[:], in0=yxv, scalar=-HALF,
                in1=zinv[:, :, None].to_broadcast([P, S3, 2]),
                op0=Alu.mult, op1=Alu.mult)

            # ---------------- main loop ------------------------------------
            for rep in range(repeat):
                for it in range(4):           # 2 windows per iter
                    op = opA if it % 2 == 0 else opB
                    osb03 = op.tile([P, C, GRP, 2], F32, tag="o03",
                                    name=f"o03_{rep}_{it}")
                    osb12 = op.tile([P, C, GRP, 2], F32, tag="o12",
                                    name=f"o12_{rep}_{it}")
                    gs = it * SW2
                    gt = gp.tile([P, SW2, GE], F16, tag="G")
                    for h in range(2):
                        w = it * 2 + h
                        # SWDGE ring holds 1024 descriptors -> sub-gathers
                        for g0 in range(0, CAP, 1024):
                            ns = min(1024, CAP - g0)
                            nc.gpsimd.dma_gather(
                                out_ap=gt[:, h * SW + g0 // P:
                                          h * SW + (g0 + ns) // P, :],
                                in_ap=featw[w][:],
                                idxs_ap=idx_sb[:, w * (CAP // 16) + g0 // 16:
                                               w * (CAP // 16) +
                                               (g0 + ns) // 16],
                                num_idxs=ns, num_idxs_reg=ns, elem_size=GE,
                                queue_num=(w * 5 + g0 // 1024) % swq)

                    lo = gt[:, :, 0:D2]            # g00|g10
                    hi = gt[:, :, D2:2 * D2]       # g01|g11

                    # materialize tx/ty operands on the idle ACT engine
                    # (doubly-stride-0 broadcast reads are slow on DVE)
                    tye_t = ep.tile([P, SW2, 2, C], CDT, tag="tye",
                                    name=f"tye_{rep}_{it}")
                    nc.scalar.copy(tye_t[:], txy[:, gs:gs + SW2, 0:1]
                                   .to_broadcast([P, SW2, 2, C]))
                    tyb = tye_t[:]
                    txe_t = ep.tile([P, SW2, 2, C], CDT, tag="txe",
                                    name=f"txe_{rep}_{it}")
                    nc.scalar.copy(txe_t[:], txy[:, gs:gs + SW2, 1:2]
                                   .to_broadcast([P, SW2, 2, C]))
                    txb = txe_t[:]
                    ayxb = ayx[:, gs:gs + SW2, :, None].to_broadcast(
                        [P, SW2, 2, C])
                    czyxb = czyx[:, gs:gs + SW2, :, None].to_broadcast(
                        [P, SW2, 2, C])

                    V = tp.tile([P, SW2, 4, C], CDT, name=f"V_{rep}_{it}",
                                tag="V")
                    # dd = hi - lo -> V[2:4]
                    nc.vector.tensor_tensor(
                        out=V[:, :, 2:4, :].rearrange("p n u c -> p n (u c)"),
                        in0=hi, in1=lo, op=Alu.subtract)
                    t_t = tp.tile([P, SW2, 2, C], CDT, name=f"tt_{rep}_{it}",
                                  tag="t_t")
                    nc.vector.tensor_tensor(out=t_t[:], in0=V[:, :, 2:4, :],
                                            in1=txb, op=Alu.mult)
                    # ab = lo + tx*dd -> V[0:2]
                    nc.vector.tensor_tensor(
                        out=V[:, :, 0:2, :].rearrange("p n u c -> p n (u c)"),
                        in0=lo,
                        in1=t_t[:].rearrange("p n u c -> p n (u c)"),
                        op=Alu.add)
                    # [dy, e] = odd half - even half
                    de = tp.tile([P, SW2, 2, C], CDT, name=f"de_{rep}_{it}",
                                 tag="de")
                    nc.vector.tensor_tensor(out=de[:], in0=V[:, :, 1::2, :],
                                            in1=V[:, :, 0::2, :],
                                            op=Alu.subtract)
                    # [tmp, tmp2] = [dy, e] * ty
                    tt2 = tp.tile([P, SW2, 2, C], CDT, name=f"tt2_{rep}_{it}",
                                  tag="tt2")
                    nc.vector.tensor_tensor(out=tt2[:], in0=de[:], in1=tyb,
                                            op=Alu.mult)
                    # phi = ab0 + tmp -> contiguous fp16 scratch
                    po = tp.tile([P, SW2, 2, C], CDT, name=f"po_{rep}_{it}",
                                 tag="po")
                    nc.vector.tensor_tensor(
                        out=po[:, :, 0, :],
                        in0=V[:, :, 0, :], in1=tt2[:, :, 0, :], op=Alu.add)
                    # djx = dd0 + tmp2 (overwrites e)
                    nc.vector.tensor_tensor(
                        out=de[:, :, 1, :], in0=V[:, :, 2, :],
                        in1=tt2[:, :, 1, :], op=Alu.add)
                    # [yCam, xCam] contiguous, reusing tt2's buffer
                    yx_t = tp.tile([P, SW2, 2, C], CDT,
                                   name=f"yx_{rep}_{it}", tag="tt2")
                    nc.vector.tensor_tensor(
                        out=yx_t[:], in0=de[:], in1=ayxb, op=Alu.mult)
                    # [t4, t5] = [dy, djx] * [czy, czx]
                    t45 = tp.tile([P, SW2, 2, C], CDT, name=f"t45_{rep}_{it}",
                                  tag="t_t")
                    nc.vector.tensor_tensor(out=t45[:], in0=de[:], in1=czyxb,
                                            op=Alu.mult)
                    nc.vector.tensor_tensor(   # zCam
                        out=po[:, :, 1, :],
                        in0=t45[:, :, 0, :], in1=t45[:, :, 1, :], op=Alu.add)
                    # idle ACT casts fp16 scratch -> f32 staging layout
                    nc.scalar.copy(
                        osb03[:].rearrange("p c s u -> p s u c"), po[:])
                    nc.scalar.copy(
                        osb12[:].rearrange("p c s u -> p s u c"), yx_t[:])

                    for g, osb_g in ((0, osb03), (1, osb12)):
                        dview = out[g].rearrange(
                            "c (p s) u -> p c s u", p=P)[
                            :, :, it * GRP:(it + 1) * GRP, :]
                        nc.sync.dma_start(dview, osb_g[:])

    nc.compile()
    return nc


def _make_core_inputs(inp_b, grid_b, fw_b, fh_b):
    """Host-side shard/layout/indexing prep for one core."""
    feat = np.ascontiguousarray(inp_b.transpose(1, 2, 0)).astype(np.float16)
    fj1 = np.concatenate([feat[:, 1:], feat[:, -1:]], axis=1)
    feat2 = np.concatenate([feat[:-1], feat[1:], fj1[:-1], fj1[1:]],
                           axis=2).reshape((H - 1) * W, GE)
    # pad to NW*WROWS rows so every window tensor is full-shape
    feat2 = np.concatenate(
        [feat2, np.zeros((NW * WROWS - feat2.shape[0], GE), np.float16)])

    jx = (grid_b[:, 0].astype(np.float32) + np.float32(1.0)) \
        * np.float32(HALF)
    iy = (grid_b[:, 1].astype(np.float32) + np.float32(1.0)) \
        * np.float32(HALF)
    j0 = np.floor(jx).astype(np.int64)
    i0 = np.floor(iy).astype(np.int64)
    row = i0 * W + j0
    order = np.argsort(row, kind="stable")
    row_s = row[order]
    bnd = np.searchsorted(row_s, np.arange(1, NW) * WROWS)
    bnd = np.concatenate([[0], bnd, [QC]])

    grid_dev_sorted = np.empty((QCP, 3), np.float32)
    idx_local = np.empty(QCP, np.int64)
    qid_sorted = np.empty(QCP, np.int64)
    for w in range(NW):
        lo, hi = bnd[w], bnd[w + 1]
        nwq = hi - lo
        assert nwq <= CAP, f"window {w} overflow: {nwq} > {CAP}"
        sl = slice(w * CAP, w * CAP + nwq)
        sel = order[lo:hi]
        grid_dev_sorted[sl] = grid_b[sel]
        idx_local[sl] = row_s[lo:hi] - w * WROWS
        qid_sorted[sl] = sel
        pad = slice(w * CAP + nwq, (w + 1) * CAP)
        if nwq > 0:
            grid_dev_sorted[pad] = grid_b[sel[-1]]
            idx_local[pad] = row_s[hi - 1] - w * WROWS
        else:
            grid_dev_sorted[pad] = grid_b[0]
            idx_local[pad] = 0
        qid_sorted[pad] = -1
    assert (idx_local >= 0).all() and (idx_local < WROWS).all()

    pp, ss = np.meshgrid(np.arange(P), np.arange(S3), indexing="ij")
    wds, kds = ss // SW, ss % SW
    spos = (wds * CAP + kds * P + pp).ravel()
    grid_dev = grid_dev_sorted[spos]
    qid = qid_sorted[spos]

    idx16 = np.empty((16, QCP // 16), np.int16)
    for w in range(NW):
        wrap = idx_local[w * CAP:(w + 1) * CAP].astype(
            np.int16).reshape(CAP // 16, 16).T
        idx16[:, w * (CAP // 16):(w + 1) * (CAP // 16)] = wrap
    idx16 = np.tile(idx16, (8, 1))

    m = {
        "grid_q": np.ascontiguousarray(grid_dev[:, [1, 0, 2]],
                                       dtype=np.float32),
        "idx16": np.ascontiguousarray(idx16),
        "fswh": np.array([[fh_b, fw_b]], dtype=np.float32),
    }
    for w in range(NW):
        m[f"featw{w}"] = np.ascontiguousarray(
            feat2[w * WROWS:(w + 1) * WROWS])
    return m, qid


_CACHED_NC = None


def kernel(input, grid, fScaleWidth, fScaleHeight):
    global _CACHED_NC
    input = np.ascontiguousarray(input, dtype=np.float32)
    grid = np.ascontiguousarray(grid, dtype=np.float32)
    fScaleWidth = np.asarray(fScaleWidth, dtype=np.float32)
    fScaleHeight = np.asarray(fScaleHeight, dtype=np.float32)

    if _CACHED_NC is None:
        _CACHED_NC = _build_kernel()
    nc = _CACHED_NC

    in_maps, qids = [], []
    for core in range(NCORES):
        b, half = core // 2, core % 2
        m, qid = _make_core_inputs(
            input[b], grid[b, half * QC:(half + 1) * QC],
            fScaleWidth[b], fScaleHeight[b])
        in_maps.append(m)
        qids.append(qid)

    from concourse import bass_utils
    res = bass_utils.run_bass_kernel_spmd(
        nc, in_maps, core_ids=list(range(NCORES)))

    output = np.empty((B, C, 4, Q), np.float32)
    for core in range(NCORES):
        b, half = core // 2, core % 2
        sl = output[b, :, :, half * QC:(half + 1) * QC]
        qid = qids[core]
        valid = qid >= 0
        o2 = res.results[core]["out"]        # [2, C, QCP, 2]
        for k, (g, u) in enumerate(((0, 0), (1, 1), (1, 0), (0, 1))):
            sl[:, k, qid[valid]] = o2[g, :, valid, u].T
    return output
